# revision 1
# baseline (speedup 1.0000x reference)
"""Trainium2 Bass kernel for batched multi-head attention.

Problem: q, k, v: [B=4, H=16, D=64, N=2048] fp32, layout (b, h, d, n).
    sim  = einsum('bhdi,bhdj->bhij', q * D**-0.5, k)
    attn = softmax(sim, axis=-1)
    out  = einsum('bhij,bhdj->bhdi', attn, v)

Sharding: the 64 (b, h) pairs split across 8 NeuronCores, 8 heads per core
(4 pairs of 2 heads stacked on 128 partitions). No collectives.

Per-core algorithm (flash-style, no max subtraction; logits are O(1)):
  stream of 512 units (pair, sub-head, i-tile of 512, j-chunk of 128),
  grouped 2 units per 2-bank PSUM slot (3 slots rotating):
    S^T[j, i] = K_chunk^T Q_tile          (fp32r matmul, 512 rows)
    expS      = exp(S * scale) -> bf16    (ACT or DVE, see below)
    O^T[i, d|sum] += expS_chunk^T [V^T|1] (bf16 matmuls, transposed-PV: output
                                           free dim is 65 so 4 matmuls per
                                           unit, accumulated over 16 j-chunks;
                                           bf16 because fp32r is 4x slower
                                           under 256 output rows)
    normalize: ACT copies the accumulator PSUM->SBUF, Pool normalize_recip
               divides by the ones-column sums, DMA out.
  V^T arrives pre-transposed from the host with a fused ones column; the
  output is written as out^T [h, n, d] tiles and un-transposed on the host.
  PSUM matmul note: start=True zeroes the WHOLE bank, so only the first
  chain of each accumulator bank sets it.

Engine split of the 256 exp groups (PE is the roofline at ~165us/core;
exp must be split so neither ACT nor DVE exceeds that):
  ACT:  exact exp activation, bf16 out            (128 groups, ~149us)
  DVE:  single-seed Schraudolph bit-trick in ONE tensor_scalar:
        int16 z = S*(scale*log2e*128) + (127*128 - 7), truncating convert,
        bitcast bf16  (128 groups, ~153us; sawtooth rel err ~1.8% rms on
        half the softmax weights -> ~1.3e-2 output rel err, tolerance 2e-2)
"""

import numpy as np
import ml_dtypes

import concourse.bass as bass
import concourse.mybir as mybir
import concourse.tile as tile
from concourse import bacc
from concourse.bass_utils import run_bass_kernel_spmd

B, H, D, N = 4, 16, 64, 2048
NCORES = 8
HPC = (B * H) // NCORES  # heads per core = 8
NPAIRS = HPC // 2        # head pairs per core = 4
ITILE = 512              # query columns per i-tile
NIT = N // ITILE         # 4 i-tiles
JC = 128                 # key chunk (contraction partition dim)
NCH = N // JC            # 16 chunks
SCALE = float(D) ** -0.5
GW = 2                   # units per group (2-bank PSUM slots)
NGROUPS = (NPAIRS * 2 * NIT * NCH) // GW  # 256

# Schraudolph constants (bf16/int16 bit trick), tuned numerically against the
# exact softmax: z = S * A + B, truncating fp32->int16 convert.
LOG2E = 1.4426950408889634
EXP_A = float(SCALE * LOG2E * 128.0)
EXP_B2 = float(127 * 128 - np.log2(1.0 + np.sqrt(2.0)) * 128.0 - 7.0)  # two-seed
EXP_B1 = float(127 * 128 - 7.0)                                        # one-seed

# exp engine assignment: counts over the 256 groups
N_DVE1 = 128  # single-seed Schraudolph on DVE
N_DVE2 = 0    # two-seed Schraudolph on DVE (slower overall: serial DVE chain)
RAMP_ACT = 0  # no forced-ACT ramp (parallel-first DMA made it moot)
TAIL_ACT = 0  # tail alternates engines (parallel X drain)
O_LAG = 6     # groups between exp(g) and its PV consumption O(g)
N_SPLIT_TAIL = 0  # split-X tail measured slower; disabled
USE_RINGS = False  # multi-ring pair-0 DMA measured slower; disabled
NORM_COPY = "act"  # which engine copies the PSUM accumulator to SBUF
NORM_HOLD = 0  # defer norm flushes in the last K groups

F32 = mybir.dt.float32
F32R = mybir.dt.float32r
BF16 = mybir.dt.bfloat16
I16 = mybir.dt.int16

_CACHE = {}


def assign_engines():
    """Per-group engine: 'A' (ACT exact exp), 'D' (DVE 1-seed Schraudolph),
    'E' (DVE 2-seed Schraudolph)."""
    eng = ["A"] * NGROUPS
    free = list(range(RAMP_ACT, NGROUPS - TAIL_ACT - N_SPLIT_TAIL))
    nfree = len(free)
    acc_d = acc_e = 0.0
    for g in free:
        acc_d += N_DVE1 / nfree
        acc_e += N_DVE2 / nfree
        if acc_e >= 1.0:
            eng[g] = "E"
            acc_e -= 1.0
        elif acc_d >= 1.0:
            eng[g] = "D"
            acc_d -= 1.0
    for i, g in enumerate(range(NGROUPS - TAIL_ACT - N_SPLIT_TAIL, NGROUPS - TAIL_ACT)):
        eng[g] = "S"
    return eng


def build_bass():
    nc = bacc.Bacc("TRN2", target_bir_lowering=False)
    qk_h = nc.dram_tensor("qk", [NPAIRS, 2, 128, N], F32, kind="ExternalInput")
    vt_h = nc.dram_tensor("vt", [NPAIRS, 128, 2 * NCH * 65], BF16, kind="ExternalInput")
    o_h = nc.dram_tensor("out", [HPC, NIT, 128, 4 * 64], F32, kind="ExternalOutput")

    qk_d = qk_h[:, :, :, :].rearrange("p t a n -> p a t n")  # [NPAIRS, 128, 2, N]

    eng = assign_engines()
    units = [
        (p, sub, it, c)
        for p in range(NPAIRS)
        for sub in range(2)
        for it in range(NIT)
        for c in range(NCH)
    ]
    groups = [units[i : i + GW] for i in range(0, len(units), GW)]
    assert len(groups) == NGROUPS

    with tile.TileContext(nc) as tc:
        with (
            tc.tile_pool(name="consts", bufs=1) as consts,
            tc.tile_pool(name="pairs", bufs=2) as pairs,
            tc.tile_pool(name="vtp", bufs=2) as vtp,
            tc.tile_pool(name="spsum", bufs=3, space="PSUM") as spsum,
            tc.tile_pool(name="opsum", bufs=2, space="PSUM") as opsum,
            tc.tile_pool(name="expp", bufs=24) as expp,
            tc.tile_pool(name="zp", bufs=24) as zp,
            tc.tile_pool(name="outp", bufs=4) as outp,
        ):
            # dummy exp so the ACT table load lands at t=0 on real hw
            dum = consts.tile([1, 8], F32, tag="dum")
            nc.vector.memset(dum, 0.0)
            nc.scalar.activation(out=dum, in_=dum, func=mybir.ActivationFunctionType.Exp)
            # PE warm-up at t=0: starts the p-state ramp clock early
            wup = consts.tile([128, 64], BF16, tag="wup")
            nc.vector.memset(wup, 0.0)
            wps = opsum.tile([128, 260], F32, tag="o", name="warm")
            nc.tensor.matmul(out=wps[0:64, 0:64], lhsT=wup, rhs=wup, start=True, stop=True)

            pair_ctx: list[dict | None] = [None] * NPAIRS

            def emit_pair_dma(p):
                qk = pairs.tile([128, 2, N], F32R, tag="qk", name=f"qk_{p}")
                src = qk_d[p].bitcast(F32R)
                vt = vtp.tile([128, 2, NCH, 65], BF16, tag="vt", name=f"vt_{p}")
                late_eng = nc.gpsimd if USE_RINGS else nc.sync
                if p == 0:
                    # ramp-critical pieces stay on the low-latency SP ring,
                    # in dependency order; late pieces ride the Pool SWDGE
                    # ring in parallel so the SP wire keeps up with the
                    # group stream
                    vtf = vt.rearrange("a b c d -> a (b c d)")
                    # k[0:256] rides the ACT ring in PARALLEL with q[0:128]
                    # on SP: the first S matmul starts ~1us earlier (ACT has
                    # no exp work this early)
                    nc.scalar.dma_start(out=qk[:, 1, 0:256], in_=src[:, 1, 0:256])
                    nc.sync.dma_start(out=qk[:, 0, 0:128], in_=src[:, 0, 0:128])
                    nc.sync.dma_start(out=qk[:, 0, 128:ITILE], in_=src[:, 0, 128:ITILE])
                    nc.sync.dma_start(out=qk[:, 1, 256:768], in_=src[:, 1, 256:768])
                    nc.sync.dma_start(out=vtf[:, 0 : 8 * 65], in_=vt_h[p][:, 0 : 8 * 65])
                    nc.sync.dma_start(out=qk[:, 1, 768:1536], in_=src[:, 1, 768:1536])
                    nc.sync.dma_start(out=vtf[:, 8 * 65 : 2 * NCH * 65], in_=vt_h[p][:, 8 * 65 : 2 * NCH * 65])
                    nc.sync.dma_start(out=qk[:, 1, 1536:N], in_=src[:, 1, 1536:N])
                    late_eng.dma_start(out=qk[:, 0, ITILE:1024], in_=src[:, 0, ITILE:1024])
                    late_eng.dma_start(out=qk[:, 0, 1024:N], in_=src[:, 0, 1024:N])
                else:
                    late_eng.dma_start(out=qk[:, 0:1, :], in_=src[:, 0:1, :])
                    late_eng.dma_start(out=qk[:, 1:2, :], in_=src[:, 1:2, :])
                    late_eng.dma_start(
                        out=vt.rearrange("a b c d -> a (b c d)"), in_=vt_h[p]
                    )
                pair_ctx[p] = {"qk": qk, "vt": vt}

            slots: dict[int, bass.AP] = {}
            exs: dict[int, bass.AP] = {}
            o_accs: dict[tuple, bass.AP] = {}
            pending_norms: list[tuple] = []

            def emit_S(g):
                slot = spsum.tile([128, GW * ITILE], F32, tag="s", name=f"s_{g}")
                for kk, (p, sub, it, c) in enumerate(groups[g]):
                    if sub == 0 and it == 0 and c == 0:
                        if p == 0:
                            emit_pair_dma(0)
                        if p + 1 < NPAIRS:
                            emit_pair_dma(p + 1)
                    qk = pair_ctx[p]["qk"]
                    hb = sub * D
                    if g == 0 and kk == 0:
                        # ramp: sub-matmuls per 128 q columns so the PE starts
                        # as soon as the first 64KB of q/k have landed (the
                        # first sub's start=True zeroes the whole bank)
                        for j in range(4):
                            nc.tensor.matmul(
                                out=slot[:, j * 128 : (j + 1) * 128],
                                lhsT=qk[hb : hb + D, 1, c * JC : (c + 1) * JC],
                                rhs=qk[hb : hb + D, 0, j * 128 : (j + 1) * 128],
                                start=(j == 0),
                                stop=True,
                            )
                    else:
                        nc.tensor.matmul(
                            out=slot[:, kk * ITILE : (kk + 1) * ITILE],
                            lhsT=qk[hb : hb + D, 1, c * JC : (c + 1) * JC],
                            rhs=qk[hb : hb + D, 0, it * ITILE : (it + 1) * ITILE],
                            start=True,
                            stop=True,
                        )
                slots[g] = slot

            def emit_X(g):
                slot = slots.pop(g)
                w = GW * ITILE
                e = eng[g]
                if e == "A":
                    ex = expp.tile([128, w], BF16, tag="exA", name=f"xa_{g}")
                    nc.scalar.activation(
                        out=ex, in_=slot, func=mybir.ActivationFunctionType.Exp,
                        scale=SCALE,
                    )
                elif e == "S":
                    # split-X: ACT does the exact exp on the first half while
                    # DVE Schraudolphs the second half, in parallel
                    exi = zp.tile([128, w], I16, tag="exS", name=f"xs_{g}")
                    nc.scalar.activation(
                        out=exi[:, 0 : w // 2].bitcast(BF16),
                        in_=slot[:, 0 : w // 2],
                        func=mybir.ActivationFunctionType.Exp,
                        scale=SCALE,
                    )
                    nc.vector.tensor_scalar(
                        out=exi[:, w // 2 : w], in0=slot[:, w // 2 : w],
                        scalar1=EXP_A, scalar2=EXP_B1,
                        op0=mybir.AluOpType.mult, op1=mybir.AluOpType.add,
                    )
                    ex = exi.bitcast(BF16)
                elif e == "D":  # DVE single-seed Schraudolph
                    z1 = zp.tile([128, w], I16, tag="z1", name=f"z1_{g}")
                    nc.vector.tensor_scalar(
                        out=z1, in0=slot, scalar1=EXP_A, scalar2=EXP_B1,
                        op0=mybir.AluOpType.mult, op1=mybir.AluOpType.add,
                    )
                    ex = z1.bitcast(BF16)
                else:  # DVE two-seed Schraudolph (phase-averaged)
                    z1 = zp.tile([128, w], I16, tag="z1", name=f"z1_{g}")
                    nc.vector.tensor_scalar(
                        out=z1, in0=slot, scalar1=EXP_A, scalar2=EXP_B2,
                        op0=mybir.AluOpType.mult, op1=mybir.AluOpType.add,
                    )
                    z2 = zp.tile([128, w], I16, tag="z2", name=f"z2_{g}")
                    nc.vector.tensor_scalar(
                        out=z2, in0=z1, scalar1=64, scalar2=None,
                        op0=mybir.AluOpType.add,
                    )
                    ex = expp.tile([128, w], BF16, tag="exE", name=f"xe_{g}")
                    nc.vector.tensor_tensor(
                        out=ex, in0=z1.bitcast(BF16), in1=z2.bitcast(BF16),
                        op=mybir.AluOpType.add,
                    )
                exs[g] = ex

            def emit_O(g):
                ex = exs.pop(g)
                for kk, (p, sub, it, c) in enumerate(groups[g]):
                    vt = pair_ctx[p]["vt"]
                    if c == 0:
                        o_accs[(p, sub, it)] = opsum.tile(
                            [128, 4 * 65], F32, tag="o", name=f"oa_{g}_{kk}"
                        )
                    acc = o_accs[(p, sub, it)]
                    for si in range(4):
                        # start=True zeroes the WHOLE psum bank, so only the
                        # very first matmul of this accumulator may set it;
                        # the other si chains accumulate onto the zeroed bank.
                        nc.tensor.matmul(
                            out=acc[:, si * 65 : (si + 1) * 65],
                            lhsT=ex[:, kk * ITILE + si * 128 : kk * ITILE + (si + 1) * 128],
                            rhs=vt[:, sub, c, :],
                            start=(c == 0 and si == 0),
                            stop=(c == NCH - 1),
                        )
                    if c == NCH - 1:
                        pending_norms.append((p, sub, it))

            def emit_norm(p, sub, it):
                # Pool (gpsimd) cannot read PSUM: ACT copies the accumulator
                # to SBUF in one op, Pool does the four divides from there.
                acc = o_accs.pop((p, sub, it))
                acc_sb = outp.tile([128, 4 * 65], F32, tag="accsb", name=f"ac_{p}_{sub}_{it}")
                use_act = NORM_COPY == "act" or (
                    NORM_COPY == "alt" and (p * 2 + sub + it) % 2 == 0
                )
                if use_act:
                    nc.scalar.activation(
                        out=acc_sb, in_=acc, func=mybir.ActivationFunctionType.Copy
                    )
                else:
                    nc.vector.tensor_copy(out=acc_sb, in_=acc)
                out_sb = outp.tile([128, 4, 64], F32, tag="osb", name=f"ot_{p}_{sub}_{it}")
                for si in range(4):
                    nc.gpsimd.normalize_recip(
                        out_sb[:, si, :],
                        acc_sb[:, si * 65 : si * 65 + 64],
                        acc_sb[:, si * 65 + 64 : si * 65 + 65],
                    )
                nc.sync.dma_start(
                    out=o_h[2 * p + sub, it],
                    in_=out_sb.rearrange("a b c -> a (b c)"),
                )

            def flush_norms(g):
                if g >= NGROUPS - NORM_HOLD:
                    return
                while pending_norms:
                    emit_norm(*pending_norms.pop(0))

            # software-pipelined emission: PE stream ... S(g+1) O(g-1) S(g+2) ...
            emit_S(0)
            for g in range(NGROUPS):
                emit_X(g)
                if g + 1 < NGROUPS:
                    emit_S(g + 1)
                if g >= O_LAG:
                    emit_O(g - O_LAG)
                flush_norms(g)
            for g in range(NGROUPS - O_LAG, NGROUPS):
                emit_O(g)
            while pending_norms:
                emit_norm(*pending_norms.pop(0))

    nc.compile()
    return nc


def pack_qk(q, k, c):
    qr = q.reshape(B * H, D, N)[c * HPC : (c + 1) * HPC].reshape(NPAIRS, 128, N)
    kr = k.reshape(B * H, D, N)[c * HPC : (c + 1) * HPC].reshape(NPAIRS, 128, N)
    return np.ascontiguousarray(np.stack([qr, kr], axis=1))  # [NPAIRS, 2, 128, N]


def pack_vt(v, c):
    vr = v.reshape(B * H, D, N)[c * HPC : (c + 1) * HPC]  # [8, 64, 2048]
    vr = vr.reshape(NPAIRS, 2, D, NCH, JC)
    # vt[p, j, s, ch, d] = v[pair p, head s, d, ch*128+j]
    vt = np.empty((NPAIRS, JC, 2, NCH, 65), dtype=np.float32)
    vt[..., 0:64] = vr.transpose(0, 4, 1, 3, 2)
    vt[..., 64] = 1.0
    return np.ascontiguousarray(
        vt.reshape(NPAIRS, JC, 2 * NCH * 65).astype(ml_dtypes.bfloat16)
    )


def kernel(q: np.ndarray, k: np.ndarray, v: np.ndarray) -> np.ndarray:
    q = np.asarray(q, dtype=np.float32)
    k = np.asarray(k, dtype=np.float32)
    v = np.asarray(v, dtype=np.float32)
    if "nc" not in _CACHE:
        _CACHE["nc"] = build_bass()
    nc = _CACHE["nc"]

    in_maps = [
        {"qk": pack_qk(q, k, c), "vt": pack_vt(v, c)} for c in range(NCORES)
    ]
    res = run_bass_kernel_spmd(nc, in_maps, core_ids=list(range(NCORES)))
    outs = []
    for c in range(NCORES):
        ot = res.results[c]["out"].reshape(HPC, NIT, 128, 4, 64)
        outs.append(ot.transpose(0, 4, 1, 3, 2).reshape(HPC, D, N))
    out = np.concatenate(outs, axis=0)
    return out.reshape(B, H, D, N).astype(np.float32)


if __name__ == "__main__":
    rng = np.random.default_rng(0)
    q = rng.standard_normal((B, H, D, N), dtype=np.float32)
    k = rng.standard_normal((B, H, D, N), dtype=np.float32)
    v = rng.standard_normal((B, H, D, N), dtype=np.float32)
    out = kernel(q, k, v)
    s = np.einsum("hdi,hdj->hij", q.reshape(-1, D, N)[:2] * SCALE, k.reshape(-1, D, N)[:2])
    p = np.exp(s - s.max(-1, keepdims=True))
    p /= p.sum(-1, keepdims=True)
    ref = np.einsum("hij,hdj->hdi", p, v.reshape(-1, D, N)[:2])
    got = out.reshape(-1, D, N)[:2]
    print("rel err (2 heads):", np.linalg.norm(got - ref) / np.linalg.norm(ref))



# revision 3
# speedup vs baseline: 1.0090x; 1.0090x over previous
"""Trainium2 Bass kernel for batched multi-head attention.

Problem: q, k, v: [B=4, H=16, D=64, N=2048] fp32, layout (b, h, d, n).
    sim  = einsum('bhdi,bhdj->bhij', q * D**-0.5, k)
    attn = softmax(sim, axis=-1)
    out  = einsum('bhij,bhdj->bhdi', attn, v)

Sharding: the 64 (b, h) pairs split across 8 NeuronCores, 8 heads per core
(4 pairs of 2 heads stacked on 128 partitions). No collectives.

Per-core algorithm (flash-style, no max subtraction; logits are O(1)):
  stream of 512 units (pair, sub-head, i-tile of 512, j-chunk of 128),
  grouped 2 units per 2-bank PSUM slot (3 slots rotating):
    S^T[j, i] = K_chunk^T Q_tile          (bf16 matmul, 512 rows; q/k are
                                           cast to bf16 on the host)
    expS      = exp(S * scale) -> bf16    (ACT or DVE, see below)
    O^T[i, d|sum] += expS_chunk^T [V^T|1] (bf16 matmuls, transposed-PV: output
                                           free dim is 65 so 4 matmuls per
                                           unit, accumulated over 16 j-chunks)
    normalize: ACT copies the accumulator PSUM->SBUF, Pool normalize_recip
               divides by the ones-column sums, DMA out. The LAST tile
               instead normalizes on DVE (reciprocal + tensor_scalar mults
               straight out of PSUM) to shorten the post-PE tail.
  V^T arrives pre-transposed from the host with a fused ones column; the
  output is written as out^T [h, n, d] tiles and un-transposed on the host.
  PSUM matmul note: start=True zeroes the WHOLE bank, so only the first
  chain of each accumulator bank sets it.

Engine split of the 256 exp groups (PE is the roofline at ~165us/core):
  ACT:  exact exp activation, bf16 out            (128 groups, ~147us)
  DVE:  single-seed Schraudolph bit-trick in ONE tensor_scalar:
        int16 z = S*(scale*log2e*128) + (127*128 - 7), truncating convert,
        bitcast bf16  (128 groups, ~153us; sawtooth rel err ~1.8% rms on
        half the softmax weights -> ~1.3e-2 output rel err, tolerance 2e-2)
  The last group's exp is split into 4 slivers alternating DVE/ACT so the
  final PV can start ~0.5us after the last S matmul.

Schedule (timeline cost model): PE busy 164.7us is the roofline. Start:
one merged qk[:, :, 0:512] DMA puts the first S at ~3.6us. Tail: DVE
norm + single out DMA ~4.4us after the last PV.
"""

import os

import numpy as np
import ml_dtypes

import concourse.bass as bass
import concourse.mybir as mybir
import concourse.tile as tile
from concourse import bacc
from concourse.bass_utils import run_bass_kernel_spmd

B, H, D, N = 4, 16, 64, 2048
NCORES = 8
HPC = (B * H) // NCORES  # heads per core = 8
NPAIRS = HPC // 2        # head pairs per core = 4
ITILE = 512              # query columns per i-tile
NIT = N // ITILE         # 4 i-tiles
JC = 128                 # key chunk (contraction partition dim)
NCH = N // JC            # 16 chunks
SCALE = float(D) ** -0.5
GW = 2                   # units per group (2-bank PSUM slots)
NUNITS = NPAIRS * 2 * NIT * NCH  # 512

# Schraudolph constants (bf16/int16 bit trick), tuned numerically against the
# exact softmax: z = S * A + B, truncating fp32->int16 convert.
LOG2E = 1.4426950408889634
EXP_A = float(SCALE * LOG2E * 128.0)
EXP_B1 = float(127 * 128 - 7.0)  # one-seed

# exp engine assignment (env overrides for schedule tuning)
def _env(name, default):
    return int(os.environ.get(name, default))

N_DVE1 = _env("K_NDVE", 124)   # single-seed Schraudolph on DVE (non-tail)
TAIL_K = _env("K_TAILK", 8)    # final GW2 groups exp'd split DVE+ACT
TAIL_DVW = _env("K_DVW", 512)  # columns of a split GW2 group on DVE (rest ACT)
N_GW1 = _env("K_NGW1", 2)      # final single-unit groups (512 cols)
GW1_MODE = _env("K_GW1MODE", 1)  # 0: split 256/256, 1: DVE-full, 2: alt D/A
HEAD_K = _env("K_HEADK", 0)    # first HEAD_K groups also split-X (fast PSUM
                               # slot recycling while DMA is the bottleneck)
O_LAG = _env("K_OLAG", 8)      # groups between exp(g) and its PV O(g)
HOLD_FROM = _env("K_HOLD", 10)  # defer norm flushes in the last HOLD_FROM groups
N_DVE_NORM = _env("K_DVENORM", 1)  # final tiles normalized via DVE divide
SPLIT_X0 = _env("K_SPLITX0", 0)  # split the first SPLIT_X0 exps (ramp bubble)
DVE_COPY_FROM = _env("K_DVECOPY", 0)  # tail tiles whose norm copy runs on DVE

# the last N_GW1 units form their own single-unit groups so the final exps
# (and so the final PVs + norm + out-DMA) have the shortest possible chain
NGROUPS = (NUNITS - N_GW1) // GW + N_GW1

F32 = mybir.dt.float32
BF16 = mybir.dt.bfloat16
I16 = mybir.dt.int16

_CACHE = {}


def assign_engines():
    """Per-group engine: 'A' (ACT exact exp), 'D' (DVE 1-seed Schraudolph),
    'S' (split across both, DVE low cols / ACT high cols)."""
    eng = ["A"] * NGROUPS
    free = list(range(HEAD_K, NGROUPS - TAIL_K - N_GW1))
    nfree = len(free)
    acc_d = 0.0
    for g in free:
        acc_d += N_DVE1 / nfree
        if acc_d >= 1.0:
            eng[g] = "D"
            acc_d -= 1.0
    for g in range(HEAD_K):
        eng[g] = "S"
    for g in range(NGROUPS - TAIL_K - N_GW1, NGROUPS):
        eng[g] = "S"
    if SPLIT_X0:
        # split the first exps WITHOUT shifting the A/D parity of the rest:
        # the PSUM S-slot ring (3 deep) recycles through X(g-3), so the
        # first full-size exps insert a one-time ~0.5us bubble at the ramp
        for g in range(SPLIT_X0):
            eng[g] = "S"
    return eng


def build_bass():
    nc = bacc.Bacc("TRN2", target_bir_lowering=False)
    # host-packed per pair as [q_it0 (512) | k (2048) | q_it1..3 (1536)] so
    # the first DMA is one contiguous run covering exactly the first S group
    qk_h = nc.dram_tensor("qk", [NPAIRS, 128, 2 * N], BF16, kind="ExternalInput")
    vt_h = nc.dram_tensor("vt", [NPAIRS, 128, 2 * NCH * 65], BF16, kind="ExternalInput")
    o_h = nc.dram_tensor("out", [HPC, NIT, 128, 4 * 64], F32, kind="ExternalOutput")

    eng = assign_engines()
    units = [
        (p, sub, it, c)
        for p in range(NPAIRS)
        for sub in range(2)
        for it in range(NIT)
        for c in range(NCH)
    ]
    groups = [units[i : i + GW] for i in range(0, len(units) - N_GW1, GW)]
    groups += [[u] for u in units[len(units) - N_GW1 :]]
    assert len(groups) == NGROUPS

    with tile.TileContext(nc) as tc:
        with (
            tc.tile_pool(name="consts", bufs=1) as consts,
            tc.tile_pool(name="pairs", bufs=2) as pairs,
            tc.tile_pool(name="vtp", bufs=2) as vtp,
            tc.tile_pool(name="spsum", bufs=3, space="PSUM") as spsum,
            tc.tile_pool(name="opsum", bufs=2, space="PSUM") as opsum,
            tc.tile_pool(name="expp", bufs=24) as expp,
            tc.tile_pool(name="zp", bufs=24) as zp,
            tc.tile_pool(name="outp", bufs=4) as outp,
        ):
            # dummy exp so the ACT table load lands at t=0 on real hw
            dum = consts.tile([1, 8], F32, tag="dum")
            nc.vector.memset(dum, 0.0)
            nc.scalar.activation(out=dum, in_=dum, func=mybir.ActivationFunctionType.Exp)
            # PE warm-up at t=0: starts the p-state ramp clock early
            wup = consts.tile([128, 64], BF16, tag="wup")
            nc.vector.memset(wup, 0.0)
            wps = opsum.tile([128, 260], F32, tag="o", name="warm")
            nc.tensor.matmul(out=wps[0:64, 0:64], lhsT=wup, rhs=wup, start=True, stop=True)

            pair_ctx: list[dict | None] = [None] * NPAIRS

            def emit_pair_dma(p):
                qk = pairs.tile([128, 2 * N], BF16, tag="qk", name=f"qk_{p}")
                src = qk_h[p]
                vt = vtp.tile([128, 2, NCH, 65], BF16, tag="vt", name=f"vt_{p}")
                vtf = vt.rearrange("a b c d -> a (b c d)")
                if p == 0:
                    # ramp-critical order: each chunk covers ~2 groups of
                    # lookahead (per-DMA latency is ~2.2us, so small chunks
                    # can't keep up with the S ramp), vt before O(0) fires
                    # (~O_LAG groups in), q i-tiles 2..3 last
                    for lo, hi in ((0, 1024), (1024, 1792), (1792, 2560),
                                   (2560, 3072)):
                        nc.sync.dma_start(out=qk[:, lo:hi], in_=src[:, lo:hi])
                    nc.sync.dma_start(out=vtf[:, 0 : 8 * 65], in_=vt_h[p][:, 0 : 8 * 65])
                    nc.sync.dma_start(out=qk[:, 3072 : 2 * N], in_=src[:, 3072 : 2 * N])
                    nc.sync.dma_start(out=vtf[:, 8 * 65 : 2 * NCH * 65], in_=vt_h[p][:, 8 * 65 : 2 * NCH * 65])
                else:
                    nc.sync.dma_start(out=qk[:, 0:2560], in_=src[:, 0:2560])
                    nc.sync.dma_start(out=qk[:, 2560 : 2 * N], in_=src[:, 2560 : 2 * N])
                    nc.sync.dma_start(out=vtf, in_=vt_h[p])
                pair_ctx[p] = {"qk": qk, "vt": vt}

            def q_ap(qk, hb, it):
                if it == 0:
                    return qk[hb : hb + D, 0:ITILE]
                return qk[hb : hb + D, N + it * ITILE : N + (it + 1) * ITILE]

            def k_ap(qk, hb, c):
                return qk[hb : hb + D, ITILE + c * JC : ITILE + (c + 1) * JC]

            slots: dict[int, bass.AP] = {}
            exs: dict[int, object] = {}
            o_accs: dict[tuple, bass.AP] = {}
            pending_norms: list[tuple] = []

            def emit_S(g):
                slot = spsum.tile([128, GW * ITILE], F32, tag="s", name=f"s_{g}")
                for kk, (p, sub, it, c) in enumerate(groups[g]):
                    if sub == 0 and it == 0 and c == 0:
                        if p == 0:
                            emit_pair_dma(0)
                        if p + 1 < NPAIRS:
                            emit_pair_dma(p + 1)
                    qk = pair_ctx[p]["qk"]
                    hb = sub * D
                    nc.tensor.matmul(
                        out=slot[:, kk * ITILE : (kk + 1) * ITILE],
                        lhsT=k_ap(qk, hb, c),
                        rhs=q_ap(qk, hb, it),
                        start=True,
                        stop=True,
                    )
                slots[g] = slot

            def emit_X(g):
                slot = slots.pop(g)
                w = len(groups[g]) * ITILE
                slot = slot[:, 0:w]
                e = eng[g]
                if e == "A":
                    ex = expp.tile([128, w], BF16, tag="exA", name=f"xa_{g}")
                    nc.scalar.activation(
                        out=ex, in_=slot, func=mybir.ActivationFunctionType.Exp,
                        scale=SCALE,
                    )
                elif e == "D":  # DVE single-seed Schraudolph
                    z1 = zp.tile([128, w], I16, tag="z1", name=f"z1_{g}")
                    nc.vector.tensor_scalar(
                        out=z1, in0=slot, scalar1=EXP_A, scalar2=EXP_B1,
                        op0=mybir.AluOpType.mult, op1=mybir.AluOpType.add,
                    )
                    ex = z1.bitcast(BF16)
                elif e == "S" and w == ITILE and GW1_MODE != 0:
                    # single-unit tail group on one full engine
                    gi = g - (NGROUPS - N_GW1)
                    if GW1_MODE == 1 or gi % 2 == 0:
                        z1 = zp.tile([128, w], I16, tag="z1", name=f"z1_{g}")
                        nc.vector.tensor_scalar(
                            out=z1, in0=slot, scalar1=EXP_A, scalar2=EXP_B1,
                            op0=mybir.AluOpType.mult, op1=mybir.AluOpType.add,
                        )
                        ex = z1.bitcast(BF16)
                    else:
                        ex = expp.tile([128, w], BF16, tag="exA", name=f"xa_{g}")
                        nc.scalar.activation(
                            out=ex, in_=slot,
                            func=mybir.ActivationFunctionType.Exp, scale=SCALE,
                        )
                else:  # 'S': split-X — DVE takes cols [0:dvw], ACT the
                    # rest, in parallel. Separate tiles: disjoint slices of
                    # one tile would be WAW-serialized by the tile framework.
                    dvw = TAIL_DVW if w == GW * ITILE else w // 2
                    pz = zp.tile([128, dvw], I16, tag="z1", name=f"zs_{g}")
                    nc.vector.tensor_scalar(
                        out=pz, in0=slot[:, 0:dvw],
                        scalar1=EXP_A, scalar2=EXP_B1,
                        op0=mybir.AluOpType.mult, op1=mybir.AluOpType.add,
                    )
                    pa = expp.tile([128, w - dvw], BF16, tag="exA", name=f"xs_{g}")
                    nc.scalar.activation(
                        out=pa,
                        in_=slot[:, dvw:w],
                        func=mybir.ActivationFunctionType.Exp,
                        scale=SCALE,
                    )
                    ex = ("split", pz.bitcast(BF16), pa, dvw)
                exs[g] = ex

            def emit_O(g):
                ex = exs.pop(g)
                for kk, (p, sub, it, c) in enumerate(groups[g]):
                    vt = pair_ctx[p]["vt"]
                    if c == 0:
                        o_accs[(p, sub, it)] = opsum.tile(
                            [128, 4 * 65], F32, tag="o", name=f"oa_{g}_{kk}"
                        )
                    acc = o_accs[(p, sub, it)]
                    for si in range(4):
                        col = kk * ITILE + si * 128
                        if isinstance(ex, tuple):
                            _, pzd, pac, dvw = ex
                            assert dvw % 128 == 0, "split must align to PV blocks"
                            if col < dvw:
                                lhsT = pzd[:, col : col + 128]
                            else:
                                lhsT = pac[:, col - dvw : col - dvw + 128]
                        else:
                            lhsT = ex[:, col : col + 128]
                        # start=True zeroes the WHOLE psum bank, so only the
                        # very first matmul of this accumulator may set it;
                        # the other si chains accumulate onto the zeroed bank.
                        nc.tensor.matmul(
                            out=acc[:, si * 65 : (si + 1) * 65],
                            lhsT=lhsT,
                            rhs=vt[:, sub, c, :],
                            start=(c == 0 and si == 0),
                            stop=(c == NCH - 1),
                        )
                    if c == NCH - 1:
                        pending_norms.append((p, sub, it))

            n_tiles_norm = [0]

            def emit_norm(p, sub, it):
                acc = o_accs.pop((p, sub, it))
                n_tiles_norm[0] += 1
                out_sb = outp.tile([128, 4, 64], F32, tag="osb", name=f"ot_{p}_{sub}_{it}")
                if n_tiles_norm[0] > NPAIRS * 2 * NIT - N_DVE_NORM:
                    # tail tile: normalize on DVE straight out of PSUM —
                    # reciprocal of the ones-column sums, then ONE custom-DVE
                    # AFFINE_MUL_REDUCE (out = (in0*1+0)*in1) with in1 the
                    # reciprocals broadcast via a stride-0 AP. (The ISA path
                    # accepts the broadcast; InstTensorTensor does not.)
                    from concourse.dve_ops import AFFINE_MUL_REDUCE
                    accv = acc.rearrange("a (b c) -> a b c", c=65)
                    rec = outp.tile([128, 4], F32, tag="rec", name=f"rc_{p}_{sub}_{it}")
                    nc.vector.reciprocal(out=rec, in_=accv[:, :, 64])
                    rec_b = bass.AP(rec.tensor, rec.offset, list(rec.ap) + [[0, 64]])
                    nc.vector._custom_dve(
                        AFFINE_MUL_REDUCE, out=out_sb, in0=accv[:, :, 0:64],
                        in1=rec_b, s0=1.0, s1=0.0,
                    )
                else:
                    # Pool (gpsimd) cannot read PSUM: ACT copies the
                    # accumulator to SBUF in one op, Pool divides from there.
                    # Near the tail ACT is the busier engine, so the last few
                    # copies run on DVE instead.
                    acc_sb = outp.tile([128, 4 * 65], F32, tag="accsb", name=f"ac_{p}_{sub}_{it}")
                    if n_tiles_norm[0] > NPAIRS * 2 * NIT - N_DVE_NORM - DVE_COPY_FROM:
                        nc.vector.tensor_copy(out=acc_sb, in_=acc)
                    else:
                        nc.scalar.activation(
                            out=acc_sb, in_=acc, func=mybir.ActivationFunctionType.Copy
                        )
                    for si in range(4):
                        nc.gpsimd.normalize_recip(
                            out_sb[:, si, :],
                            acc_sb[:, si * 65 : si * 65 + 64],
                            acc_sb[:, si * 65 + 64 : si * 65 + 65],
                        )
                nc.sync.dma_start(
                    out=o_h[2 * p + sub, it],
                    in_=out_sb.rearrange("a b c -> a (b c)"),
                )

            def flush_norms(g):
                # defer tail-region flushes: an ACT copy wedged between
                # split-X pieces would delay the last exps (and so the last
                # PVs); the held norms drain after the final exp emission
                if g >= NGROUPS - HOLD_FROM:
                    return
                while pending_norms:
                    emit_norm(*pending_norms.pop(0))

            # software-pipelined emission: PE stream ... S(g+1) O(g-1) S(g+2) ...
            emit_S(0)
            for g in range(NGROUPS):
                emit_X(g)
                if g + 1 < NGROUPS:
                    emit_S(g + 1)
                if g >= O_LAG:
                    emit_O(g - O_LAG)
                flush_norms(g)
            for g in range(NGROUPS - O_LAG, NGROUPS):
                emit_O(g)
            while pending_norms:
                emit_norm(*pending_norms.pop(0))

    nc.compile()
    return nc


def pack_qk(q, k, c):
    qr = q.reshape(B * H, D, N)[c * HPC : (c + 1) * HPC].reshape(NPAIRS, 128, N)
    kr = k.reshape(B * H, D, N)[c * HPC : (c + 1) * HPC].reshape(NPAIRS, 128, N)
    arr = np.concatenate([qr[:, :, 0:ITILE], kr, qr[:, :, ITILE:]], axis=2)
    return np.ascontiguousarray(arr.astype(ml_dtypes.bfloat16))  # [NPAIRS, 128, 2N]


def pack_vt(v, c):
    vr = v.reshape(B * H, D, N)[c * HPC : (c + 1) * HPC]  # [8, 64, 2048]
    vr = vr.reshape(NPAIRS, 2, D, NCH, JC)
    # vt[p, j, s, ch, d] = v[pair p, head s, d, ch*128+j]
    vt = np.empty((NPAIRS, JC, 2, NCH, 65), dtype=np.float32)
    vt[..., 0:64] = vr.transpose(0, 4, 1, 3, 2)
    vt[..., 64] = 1.0
    return np.ascontiguousarray(
        vt.reshape(NPAIRS, JC, 2 * NCH * 65).astype(ml_dtypes.bfloat16)
    )


def kernel(q: np.ndarray, k: np.ndarray, v: np.ndarray) -> np.ndarray:
    q = np.asarray(q, dtype=np.float32)
    k = np.asarray(k, dtype=np.float32)
    v = np.asarray(v, dtype=np.float32)
    if "nc" not in _CACHE:
        _CACHE["nc"] = build_bass()
    nc = _CACHE["nc"]

    in_maps = [
        {"qk": pack_qk(q, k, c), "vt": pack_vt(v, c)} for c in range(NCORES)
    ]
    res = run_bass_kernel_spmd(nc, in_maps, core_ids=list(range(NCORES)))
    outs = []
    for c in range(NCORES):
        ot = res.results[c]["out"].reshape(HPC, NIT, 128, 4, 64)
        outs.append(ot.transpose(0, 4, 1, 3, 2).reshape(HPC, D, N))
    out = np.concatenate(outs, axis=0)
    return out.reshape(B, H, D, N).astype(np.float32)


if __name__ == "__main__":
    rng = np.random.default_rng(0)
    q = rng.standard_normal((B, H, D, N), dtype=np.float32)
    k = rng.standard_normal((B, H, D, N), dtype=np.float32)
    v = rng.standard_normal((B, H, D, N), dtype=np.float32)
    out = kernel(q, k, v)
    s = np.einsum("hdi,hdj->hij", q.reshape(-1, D, N)[:2] * SCALE, k.reshape(-1, D, N)[:2])
    p = np.exp(s - s.max(-1, keepdims=True))
    p /= p.sum(-1, keepdims=True)
    ref = np.einsum("hij,hdj->hdi", p, v.reshape(-1, D, N)[:2])
    got = out.reshape(-1, D, N)[:2]
    print("rel err (2 heads):", np.linalg.norm(got - ref) / np.linalg.norm(ref))


# revision 5
# speedup vs baseline: 1.0100x; 1.0010x over previous
"""Trainium2 Bass kernel for batched multi-head attention.

Problem: q, k, v: [B=4, H=16, D=64, N=2048] fp32, layout (b, h, d, n).
    sim  = einsum('bhdi,bhdj->bhij', q * D**-0.5, k)
    attn = softmax(sim, axis=-1)
    out  = einsum('bhij,bhdj->bhdi', attn, v)

Sharding: the 64 (b, h) pairs split across 8 NeuronCores, 8 heads per core
(4 pairs of 2 heads stacked on 128 partitions). No collectives.

Per-core algorithm (flash-style, no max subtraction; logits are O(1)):
  stream of 512 units (pair, sub-head, i-tile of 512, j-chunk of 128),
  grouped 2 units per 2-bank PSUM slot (3 slots rotating):
    S^T[j, i] = K_chunk^T Q_tile          (bf16 matmul, 512 rows; q/k are
                                           cast to bf16 on the host)
    expS      = exp(S * scale) -> bf16    (ACT or DVE, see below)
    O^T[i, d|sum] += expS_chunk^T [V^T|1] (bf16 matmuls, transposed-PV: output
                                           free dim is 65 so 4 matmuls per
                                           unit, accumulated over 16 j-chunks)
    normalize: ACT copies the accumulator PSUM->SBUF, Pool normalize_recip
               divides by the ones-column sums, DMA out. The LAST tile
               instead normalizes on DVE (reciprocal + tensor_scalar mults
               straight out of PSUM) to shorten the post-PE tail.
  V^T arrives pre-transposed from the host with a fused ones column; the
  output is written as out^T [h, n, d] tiles and un-transposed on the host.
  PSUM matmul note: start=True zeroes the WHOLE bank, so only the first
  chain of each accumulator bank sets it.

Engine split of the 256 exp groups (PE is the roofline at ~165us/core):
  ACT:  exact exp activation, bf16 out            (128 groups, ~147us)
  DVE:  single-seed Schraudolph bit-trick in ONE tensor_scalar:
        int16 z = S*(scale*log2e*128) + (127*128 - 7), truncating convert,
        bitcast bf16  (128 groups, ~153us; sawtooth rel err ~1.8% rms on
        half the softmax weights -> ~1.3e-2 output rel err, tolerance 2e-2)
  The last group's exp is split into 4 slivers alternating DVE/ACT so the
  final PV can start ~0.5us after the last S matmul.

Schedule (timeline cost model): PE busy 164.7us is the roofline. Start:
one merged qk[:, :, 0:512] DMA puts the first S at ~3.6us. Tail: DVE
norm + single out DMA ~4.4us after the last PV.
"""

import os

import numpy as np
import ml_dtypes

import concourse.bass as bass
import concourse.mybir as mybir
import concourse.tile as tile
from concourse import bacc
from concourse.bass_utils import run_bass_kernel_spmd

B, H, D, N = 4, 16, 64, 2048
NCORES = 8
HPC = (B * H) // NCORES  # heads per core = 8
NPAIRS = HPC // 2        # head pairs per core = 4
ITILE = 512              # query columns per i-tile
NIT = N // ITILE         # 4 i-tiles
JC = 128                 # key chunk (contraction partition dim)
NCH = N // JC            # 16 chunks
SCALE = float(D) ** -0.5
NUNITS = NPAIRS * 2 * NIT * NCH  # 512

# Schraudolph constants (bf16/int16 bit trick), tuned numerically against the
# exact softmax: z = S * A + B, truncating fp32->int16 convert.
LOG2E = 1.4426950408889634
EXP_A = float(SCALE * LOG2E * 128.0)
EXP_B1 = float(127 * 128 - 7.0)  # one-seed

# exp engine assignment (env overrides for schedule tuning)
def _env(name, default):
    return int(os.environ.get(name, default))

GW = _env("K_GW", 2)     # units per group (GW-bank PSUM slots)
SBUFS = _env("K_SBUFS", 3 if GW == 2 else 2)  # S-slot ring depth

N_DVE1 = _env("K_NDVE", 124)   # single-seed Schraudolph on DVE (non-tail)
TAIL_K = _env("K_TAILK", 8)    # final GW2 groups exp'd split DVE+ACT
TAIL_DVW = _env("K_DVW", 512)  # columns of a split GW2 group on DVE (rest ACT)
N_GW1 = _env("K_NGW1", 2)      # final single-unit groups (512 cols)
GW1_MODE = _env("K_GW1MODE", 1)  # 0: split 256/256, 1: DVE-full, 2: alt D/A
HEAD_K = _env("K_HEADK", 0)    # first HEAD_K groups also split-X (fast PSUM
                               # slot recycling while DMA is the bottleneck)
O_LAG = _env("K_OLAG", 8)      # groups between exp(g) and its PV O(g)
HOLD_FROM = _env("K_HOLD", 10)  # defer norm flushes in the last HOLD_FROM groups
N_DVE_NORM = _env("K_DVENORM", 1)  # final tiles normalized via DVE divide
SPLIT_X0 = _env("K_SPLITX0", 0)  # split the first SPLIT_X0 exps (ramp bubble)
DVE_COPY_FROM = _env("K_DVECOPY", 0)  # tail tiles whose norm copy runs on DVE
KVWB = _env("K_KVWB", 0)  # last tile out-DMA via prepped SWDGE writeback

# the last N_GW1 units form their own single-unit groups so the final exps
# (and so the final PVs + norm + out-DMA) have the shortest possible chain
NGROUPS = (NUNITS - N_GW1) // GW + N_GW1

F32 = mybir.dt.float32
BF16 = mybir.dt.bfloat16
I16 = mybir.dt.int16

_CACHE = {}


def assign_engines():
    """Per-group engine: 'A' (ACT exact exp), 'D' (DVE 1-seed Schraudolph),
    'S' (split across both, DVE low cols / ACT high cols)."""
    eng = ["A"] * NGROUPS
    free = list(range(HEAD_K, NGROUPS - TAIL_K - N_GW1))
    nfree = len(free)
    acc_d = 0.0
    for g in free:
        acc_d += N_DVE1 / nfree
        if acc_d >= 1.0:
            eng[g] = "D"
            acc_d -= 1.0
    for g in range(HEAD_K):
        eng[g] = "S"
    for g in range(NGROUPS - TAIL_K - N_GW1, NGROUPS):
        eng[g] = "S"
    if SPLIT_X0:
        # split the first exps WITHOUT shifting the A/D parity of the rest:
        # the PSUM S-slot ring (3 deep) recycles through X(g-3), so the
        # first full-size exps insert a one-time ~0.5us bubble at the ramp
        for g in range(SPLIT_X0):
            eng[g] = "S"
    for g_s in os.environ.get("K_SPLITG", "").split(","):
        if g_s.strip():
            eng[int(g_s)] = "S"
    nswap = _env("K_ESWAP", 0)
    if nswap:
        # early phase is S-heavy: DVE's 1192ns/group nearly fills its 1290ns
        # budget there while ACT has slack. Swap a few D->A early and
        # compensate A->D mid-stream to keep engine totals unchanged.
        flipped = 0
        for g in range(4, 48):
            if flipped < nswap and eng[g] == "D":
                eng[g] = "A"
                flipped += 1
        flipped = 0
        for g in range(70, 140):
            if flipped < nswap and eng[g] == "A":
                eng[g] = "D"
                flipped += 1
    return eng


def build_bass():
    nc = bacc.Bacc("TRN2", target_bir_lowering=False)
    # host-packed per pair as [q_it0 (512) | k (2048) | q_it1..3 (1536)] so
    # the first DMA is one contiguous run covering exactly the first S group
    qk_h = nc.dram_tensor("qk", [NPAIRS, 128, 2 * N], BF16, kind="ExternalInput")
    vt_h = nc.dram_tensor("vt", [NPAIRS, 128, 2 * NCH * 65], BF16, kind="ExternalInput")
    OUT_DT = BF16 if _env("K_OBF16", 1) else F32
    o_h = nc.dram_tensor("out", [HPC, NIT, 128, 4 * 64], OUT_DT, kind="ExternalOutput")

    eng = assign_engines()
    units = [
        (p, sub, it, c)
        for p in range(NPAIRS)
        for sub in range(2)
        for it in range(NIT)
        for c in range(NCH)
    ]
    groups = [units[i : i + GW] for i in range(0, len(units) - N_GW1, GW)]
    groups += [[u] for u in units[len(units) - N_GW1 :]]
    assert len(groups) == NGROUPS

    with tile.TileContext(nc) as tc:
        with (
            tc.tile_pool(name="consts", bufs=1) as consts,
            tc.tile_pool(name="pairs", bufs=2) as pairs,
            tc.tile_pool(name="vtp", bufs=2) as vtp,
            tc.tile_pool(name="spsum", bufs=SBUFS, space="PSUM") as spsum,
            tc.tile_pool(name="opsum", bufs=2, space="PSUM") as opsum,
            tc.tile_pool(name="expp", bufs=24) as expp,
            tc.tile_pool(name="zp", bufs=24) as zp,
            tc.tile_pool(name="outp", bufs=4) as outp,
            tc.tile_pool(name="lastp", bufs=1) as lastp,
        ):
            # PE warm-up ASAP: the p-state ramp clock starts at the first
            # PE matmul (removing this costs ~2.9us of mid-p-state matmuls),
            # so emit its memset first and keep the matmul 1 column wide
            wup = consts.tile([128, 64], BF16, tag="wup")
            nc.vector.memset(wup, 0.0)
            wps = opsum.tile([128, 260], F32, tag="o", name="warm")
            wcols = int(os.environ.get("K_WARMW", 1))
            nc.tensor.matmul(out=wps[0:64, 0:wcols], lhsT=wup, rhs=wup[:, 0:wcols], start=True, stop=True)
            # dummy exp so the ACT table load lands at t=0 on real hw
            dum = consts.tile([1, 8], F32, tag="dum")
            nc.vector.memset(dum, 0.0)
            nc.scalar.activation(out=dum, in_=dum, func=mybir.ActivationFunctionType.Exp)

            # last-tile writeback: descriptors prepped early on the SWDGE
            # ring (addresses only), fired by trigger_dma(count=None) after
            # the norm — the Tile-managed path defers the prep's source-read
            # dependency to the trigger, so the trigger automatically waits
            # the norm (RAW) and the prep (Pool sem). Skips the HWDGE(625)+
            # DGE(650) issue chain on the tail.
            kv_idx = consts.tile([128, 1], mybir.dt.int32, tag="kvi")
            kv_spc = consts.tile([1, 8], F32, tag="kvspc")
            nc.vector.memset(kv_idx, 0)
            kv_sem = nc.alloc_semaphore(name="kvwb_dma")
            last_out = lastp.tile([128, 4, 64], OUT_DT, tag="lo", name="last_out")

            def emit_prep():
                outf = last_out.rearrange("a b c -> a (b c)")
                in4 = bass.AP(outf.tensor, outf.offset,
                              [list(outf.ap[0]), [256, 1], [256, 1], [1, 256]])
                od = o_h[HPC - 1, NIT - 1]
                o4 = bass.AP(od.tensor, od.offset,
                             [[0, 1], [256, 128], [256, 1], [1, 256]])
                nc.gpsimd.kv_writeback(out_ap=o4, in_ap=in4,
                                       ctx_idxs_ap=kv_idx,
                                       prepare_only=True, sem=kv_sem)

            pair_ctx: list[dict | None] = [None] * NPAIRS

            def emit_pair_dma(p):
                qk = pairs.tile([128, 2 * N], BF16, tag="qk", name=f"qk_{p}")
                src = qk_h[p]
                vt = vtp.tile([128, 2, NCH, 65], BF16, tag="vt", name=f"vt_{p}")
                vtf = vt.rearrange("a b c d -> a (b c d)")
                if p == 0:
                    # ramp-critical order: each chunk covers ~2 groups of
                    # lookahead (per-DMA latency is ~2.2us, so small chunks
                    # can't keep up with the S ramp), vt before O(0) fires
                    # (~O_LAG groups in), q i-tiles 2..3 last
                    for lo, hi in ((0, 1024), (1024, 1792), (1792, 2560),
                                   (2560, 3072)):
                        nc.sync.dma_start(out=qk[:, lo:hi], in_=src[:, lo:hi])
                    nc.sync.dma_start(out=vtf[:, 0 : 8 * 65], in_=vt_h[p][:, 0 : 8 * 65])
                    nc.sync.dma_start(out=qk[:, 3072 : 2 * N], in_=src[:, 3072 : 2 * N])
                    nc.sync.dma_start(out=vtf[:, 8 * 65 : 2 * NCH * 65], in_=vt_h[p][:, 8 * 65 : 2 * NCH * 65])
                else:
                    nc.sync.dma_start(out=qk[:, 0:2560], in_=src[:, 0:2560])
                    nc.sync.dma_start(out=qk[:, 2560 : 2 * N], in_=src[:, 2560 : 2 * N])
                    nc.sync.dma_start(out=vtf, in_=vt_h[p])
                pair_ctx[p] = {"qk": qk, "vt": vt}

            def q_ap(qk, hb, it):
                if it == 0:
                    return qk[hb : hb + D, 0:ITILE]
                return qk[hb : hb + D, N + it * ITILE : N + (it + 1) * ITILE]

            def k_ap(qk, hb, c):
                return qk[hb : hb + D, ITILE + c * JC : ITILE + (c + 1) * JC]

            slots: dict[int, bass.AP] = {}
            exs: dict[int, object] = {}
            o_accs: dict[tuple, bass.AP] = {}
            pending_norms: list[tuple] = []

            def emit_S(g):
                slot = spsum.tile([128, GW * ITILE], F32, tag="s", name=f"s_{g}")
                for kk, (p, sub, it, c) in enumerate(groups[g]):
                    if sub == 0 and it == 0 and c == 0:
                        if p == 0:
                            emit_pair_dma(0)
                        if p + 1 < NPAIRS:
                            emit_pair_dma(p + 1)
                    qk = pair_ctx[p]["qk"]
                    hb = sub * D
                    nc.tensor.matmul(
                        out=slot[:, kk * ITILE : (kk + 1) * ITILE],
                        lhsT=k_ap(qk, hb, c),
                        rhs=q_ap(qk, hb, it),
                        start=True,
                        stop=True,
                    )
                slots[g] = slot

            def emit_X(g):
                slot = slots.pop(g)
                w = len(groups[g]) * ITILE
                slot = slot[:, 0:w]
                e = eng[g]
                if e == "A":
                    ex = expp.tile([128, w], BF16, tag="exA", name=f"xa_{g}")
                    nc.scalar.activation(
                        out=ex, in_=slot, func=mybir.ActivationFunctionType.Exp,
                        scale=SCALE,
                    )
                elif e == "D":  # DVE single-seed Schraudolph
                    z1 = zp.tile([128, w], I16, tag="z1", name=f"z1_{g}")
                    nc.vector.tensor_scalar(
                        out=z1, in0=slot, scalar1=EXP_A, scalar2=EXP_B1,
                        op0=mybir.AluOpType.mult, op1=mybir.AluOpType.add,
                    )
                    ex = z1.bitcast(BF16)
                elif e == "S" and w == ITILE and GW1_MODE != 0:
                    # single-unit tail group on one full engine
                    gi = g - (NGROUPS - N_GW1)
                    use_dve = {1: True, 2: gi % 2 == 0, 3: False, 4: gi % 2 == 1}[GW1_MODE]
                    if use_dve:
                        z1 = zp.tile([128, w], I16, tag="z1", name=f"z1_{g}")
                        nc.vector.tensor_scalar(
                            out=z1, in0=slot, scalar1=EXP_A, scalar2=EXP_B1,
                            op0=mybir.AluOpType.mult, op1=mybir.AluOpType.add,
                        )
                        ex = z1.bitcast(BF16)
                    else:
                        ex = expp.tile([128, w], BF16, tag="exA", name=f"xa_{g}")
                        nc.scalar.activation(
                            out=ex, in_=slot,
                            func=mybir.ActivationFunctionType.Exp, scale=SCALE,
                        )
                else:  # 'S': split-X — DVE takes cols [0:dvw], ACT the
                    # rest, in parallel. Separate tiles: disjoint slices of
                    # one tile would be WAW-serialized by the tile framework.
                    dvw = TAIL_DVW if w == GW * ITILE else w // 2
                    # first split groups: ACT is still draining its last
                    # full-group exp; shift their split DVE-ward
                    s0 = NGROUPS - TAIL_K - N_GW1
                    if w == GW * ITILE and g - s0 < _env("K_DVW0N", 0):
                        dvw = _env("K_DVW0", 640)
                    pz = zp.tile([128, dvw], I16, tag="z1", name=f"zs_{g}")
                    nc.vector.tensor_scalar(
                        out=pz, in0=slot[:, 0:dvw],
                        scalar1=EXP_A, scalar2=EXP_B1,
                        op0=mybir.AluOpType.mult, op1=mybir.AluOpType.add,
                    )
                    pa = expp.tile([128, w - dvw], BF16, tag="exA", name=f"xs_{g}")
                    nc.scalar.activation(
                        out=pa,
                        in_=slot[:, dvw:w],
                        func=mybir.ActivationFunctionType.Exp,
                        scale=SCALE,
                    )
                    ex = ("split", pz.bitcast(BF16), pa, dvw)
                exs[g] = ex

            def emit_O(g):
                ex = exs.pop(g)
                for kk, (p, sub, it, c) in enumerate(groups[g]):
                    vt = pair_ctx[p]["vt"]
                    if c == 0:
                        o_accs[(p, sub, it)] = opsum.tile(
                            [128, 4 * 65], F32, tag="o", name=f"oa_{g}_{kk}"
                        )
                    acc = o_accs[(p, sub, it)]
                    for si in range(4):
                        col = kk * ITILE + si * 128
                        if isinstance(ex, tuple):
                            _, pzd, pac, dvw = ex
                            assert dvw % 128 == 0, "split must align to PV blocks"
                            if col < dvw:
                                lhsT = pzd[:, col : col + 128]
                            else:
                                lhsT = pac[:, col - dvw : col - dvw + 128]
                        else:
                            lhsT = ex[:, col : col + 128]
                        # start=True zeroes the WHOLE psum bank, so only the
                        # very first matmul of this accumulator may set it;
                        # the other si chains accumulate onto the zeroed bank.
                        nc.tensor.matmul(
                            out=acc[:, si * 65 : (si + 1) * 65],
                            lhsT=lhsT,
                            rhs=vt[:, sub, c, :],
                            start=(c == 0 and si == 0),
                            stop=(c == NCH - 1),
                        )
                    if c == NCH - 1:
                        pending_norms.append((p, sub, it))

            n_tiles_norm = [0]

            def emit_norm(p, sub, it):
                acc = o_accs.pop((p, sub, it))
                n_tiles_norm[0] += 1
                is_last = n_tiles_norm[0] == NPAIRS * 2 * NIT
                if KVWB and is_last:
                    out_sb = last_out
                else:
                    out_sb = outp.tile([128, 4, 64], OUT_DT, tag="osb", name=f"ot_{p}_{sub}_{it}")
                if n_tiles_norm[0] > NPAIRS * 2 * NIT - N_DVE_NORM:
                    # tail tile: normalize on DVE straight out of PSUM —
                    # reciprocal of the ones-column sums, then ONE custom-DVE
                    # AFFINE_MUL_REDUCE (out = (in0*1+0)*in1) with in1 the
                    # reciprocals broadcast via a stride-0 AP. (The ISA path
                    # accepts the broadcast; InstTensorTensor does not.)
                    from concourse.dve_ops import AFFINE_MUL_REDUCE
                    accv = acc.rearrange("a (b c) -> a b c", c=65)
                    rec = outp.tile([128, 4], F32, tag="rec", name=f"rc_{p}_{sub}_{it}")
                    nc.vector.reciprocal(out=rec, in_=accv[:, :, 64])
                    rec_b = bass.AP(rec.tensor, rec.offset, list(rec.ap) + [[0, 64]])
                    nc.vector._custom_dve(
                        AFFINE_MUL_REDUCE, out=out_sb, in0=accv[:, :, 0:64],
                        in1=rec_b, s0=1.0, s1=0.0,
                    )
                else:
                    # Pool (gpsimd) cannot read PSUM: ACT copies the
                    # accumulator to SBUF in one op, Pool divides from there.
                    # Near the tail ACT is the busier engine, so the last few
                    # copies run on DVE instead.
                    acc_sb = outp.tile([128, 4 * 65], F32, tag="accsb", name=f"ac_{p}_{sub}_{it}")
                    if n_tiles_norm[0] > NPAIRS * 2 * NIT - N_DVE_NORM - DVE_COPY_FROM:
                        nc.vector.tensor_copy(out=acc_sb, in_=acc)
                    else:
                        nc.scalar.activation(
                            out=acc_sb, in_=acc, func=mybir.ActivationFunctionType.Copy
                        )
                    for si in range(4):
                        nc.gpsimd.normalize_recip(
                            out_sb[:, si, :],
                            acc_sb[:, si * 65 : si * 65 + 64],
                            acc_sb[:, si * 65 + 64 : si * 65 + 65],
                        )
                if KVWB and is_last:
                    # prep emitted AFTER the norm so its source-read dep (the
                    # custom-DVE norm write) exists at emission; the Tile-
                    # managed trigger then inherits that deferred dep while
                    # the desc-gen itself runs unconstrained. The framework's
                    # drain waits the completion sem.
                    emit_prep()
                    nc.gpsimd.trigger_dma(count=None)
                    # spacer ops: the deferred transfer track needs a brief
                    # Pool-SEQ acquisition; give it free/acquire cycles before
                    # the epilogue barrier parks holding the sequencer. The
                    # spacer tile must be UNRELATED to the prep (a shared tile
                    # would add a WAR edge against the DMA completion).
                    for _ in range(3):
                        nc.gpsimd.memset(kv_spc, 0.0)
                else:
                    nc.sync.dma_start(
                        out=o_h[2 * p + sub, it],
                        in_=out_sb.rearrange("a b c -> a (b c)"),
                    )

            def flush_norms(g):
                # defer tail-region flushes: an ACT copy wedged between
                # split-X pieces would delay the last exps (and so the last
                # PVs); the held norms drain after the final exp emission
                if g >= NGROUPS - HOLD_FROM:
                    return
                while pending_norms:
                    emit_norm(*pending_norms.pop(0))

            # software-pipelined emission: PE stream ... S(g+1) O(g-1) S(g+2) ...
            emit_S(0)
            for g in range(NGROUPS):
                emit_X(g)
                if g + 1 < NGROUPS:
                    emit_S(g + 1)
                if g >= O_LAG:
                    emit_O(g - O_LAG)
                flush_norms(g)
            for g in range(NGROUPS - O_LAG, NGROUPS):
                emit_O(g)
            while pending_norms:
                emit_norm(*pending_norms.pop(0))

    if KVWB:
        # post-pass: the Tile epilogue drain waits the DMASW0 lane sem, but
        # a prepared DMA's completion fires the user sem that kv_writeback
        # requires (OnUpdate[0], deferred to the trigger). Retarget
        # OnUpdate[0] at the lane sem so the completion signal is the one
        # the drain (and the timeline) actually observes.
        insts = []
        for blk in nc.m.functions[0].blocks:
            insts.extend(list(blk.instructions))
        prep = next(
            i for i in insts if type(i).__name__ == "InstKVWritebackAnt"
        )
        dmasw = next(
            w
            for i in insts
            if i.sync_info
            for w in (i.sync_info.on_wait or [])
            if w.ant_name and w.ant_name.startswith("DMASW")
        )
        ups = list(prep.sync_info.on_update)
        assert ups[0].ant_name == "kvwb_dma"
        ups[0] = mybir.SyncUpdate(
            sync_type="semaphore", id=dmasw.id, ant_name=dmasw.ant_name,
            update_mode="sem-inc", update_value=16, update_reg=None,
        )
        prep.sync_info.on_update = ups

    nc.compile()
    return nc


def pack_qk(q, k, c):
    qr = q.reshape(B * H, D, N)[c * HPC : (c + 1) * HPC].reshape(NPAIRS, 128, N)
    kr = k.reshape(B * H, D, N)[c * HPC : (c + 1) * HPC].reshape(NPAIRS, 128, N)
    arr = np.concatenate([qr[:, :, 0:ITILE], kr, qr[:, :, ITILE:]], axis=2)
    return np.ascontiguousarray(arr.astype(ml_dtypes.bfloat16))  # [NPAIRS, 128, 2N]


def pack_vt(v, c):
    vr = v.reshape(B * H, D, N)[c * HPC : (c + 1) * HPC]  # [8, 64, 2048]
    vr = vr.reshape(NPAIRS, 2, D, NCH, JC)
    # vt[p, j, s, ch, d] = v[pair p, head s, d, ch*128+j]
    vt = np.empty((NPAIRS, JC, 2, NCH, 65), dtype=np.float32)
    vt[..., 0:64] = vr.transpose(0, 4, 1, 3, 2)
    vt[..., 64] = 1.0
    return np.ascontiguousarray(
        vt.reshape(NPAIRS, JC, 2 * NCH * 65).astype(ml_dtypes.bfloat16)
    )


def kernel(q: np.ndarray, k: np.ndarray, v: np.ndarray) -> np.ndarray:
    q = np.asarray(q, dtype=np.float32)
    k = np.asarray(k, dtype=np.float32)
    v = np.asarray(v, dtype=np.float32)
    if "nc" not in _CACHE:
        _CACHE["nc"] = build_bass()
    nc = _CACHE["nc"]

    in_maps = [
        {"qk": pack_qk(q, k, c), "vt": pack_vt(v, c)} for c in range(NCORES)
    ]
    res = run_bass_kernel_spmd(nc, in_maps, core_ids=list(range(NCORES)))
    outs = []
    for c in range(NCORES):
        ot = np.asarray(res.results[c]["out"], dtype=np.float32).reshape(HPC, NIT, 128, 4, 64)
        outs.append(ot.transpose(0, 4, 1, 3, 2).reshape(HPC, D, N))
    out = np.concatenate(outs, axis=0)
    return out.reshape(B, H, D, N).astype(np.float32)


if __name__ == "__main__":
    rng = np.random.default_rng(0)
    q = rng.standard_normal((B, H, D, N), dtype=np.float32)
    k = rng.standard_normal((B, H, D, N), dtype=np.float32)
    v = rng.standard_normal((B, H, D, N), dtype=np.float32)
    out = kernel(q, k, v)
    s = np.einsum("hdi,hdj->hij", q.reshape(-1, D, N)[:2] * SCALE, k.reshape(-1, D, N)[:2])
    p = np.exp(s - s.max(-1, keepdims=True))
    p /= p.sum(-1, keepdims=True)
    ref = np.einsum("hij,hdj->hdi", p, v.reshape(-1, D, N)[:2])
    got = out.reshape(-1, D, N)[:2]
    print("rel err (2 heads):", np.linalg.norm(got - ref) / np.linalg.norm(ref))
